# revision 52
# baseline (speedup 1.0000x reference)
"""Multi-head self-attention (B=4, S=2048, D=1024, 16 heads x 64) on 8 TRN2
NeuronCores via Bass/Tile.

Sharding: tensor-parallel over heads. Each core owns 2 heads (128 of the 1024
Q/K/V output features, column-parallel) and the matching 128 rows of Wo
(row-parallel). Every core computes a full-shape partial output in bf16; the
host sums the 8 partials and adds bo (the row-parallel all-reduce).

Per-core dataflow (matmul operands bf16, accumulation fp32 in PSUM):
  xT[b]   : [D, S] features-on-partitions (host pre-transposed)
  qT/kT/vT: [128, S]  = (x @ W)^T per core, via lhsT=W k-tiles, rhs=xT,
            emitted as unbroken 8-matmul accumulation chains so the PE's
            LDWEIGHTS pull-ahead hides every weight load
  v_aug   : PE-transpose of vT -> v natural [S,64] per head + ones column,
            both heads repacked with a single 2-dim-AP DVE copy per j-tile
  scoresT : [j, i] per j-tile; the two heads occupy PE row-groups
            (0-63 / 64-127) and run concurrently, one [128,1024] PSUM tile
  exp     : one ACT Exp op per (i-chunk, j-tile) covering both heads
            (scale=1/8, per-partition bias = attention-mask column)
  PV      : lhsT=[v_h | ones] [128 j, 65], rhs=expT half, accumulated over
            j-tiles -> rows 0-63 ctx^T, row 64 = softmax denominator
  norm    : reciprocal_approx_fast straight from the PSUM denominator row +
            GPSIMD partition broadcast, multiply -> ctxT [128, S] bf16
  out     : lhsT=ctxT tile [128,128], rhs=Wo_c [128,512] chunks; DVE copies
            PSUM->SBUF as bf16 (no bias on device); DMA partial to DRAM

The emission is software-pipelined at chain granularity: batch b's attention
units are interleaved with batch b+1's QKV chains and batch b-1's
output-projection units, so the in-order PE always has independent matmul
work while the ACT-bound softmax stream runs, and consecutive matmuls of one
chain stay adjacent in the PE queue (no LDWEIGHTS thrash).
"""

import numpy as np
import ml_dtypes

import concourse.bass as bass
import concourse.mybir as mybir
import concourse.tile as tile
from concourse import bacc, bass_utils
from concourse.masks import make_identity

F32 = mybir.dt.float32
BF16 = mybir.dt.bfloat16
AF = mybir.ActivationFunctionType
BF = ml_dtypes.bfloat16
ts = bass.ts

B, S, D = 4, 2048, 1024
NH, HD = 16, 64
NCORES = 8
OF = D // NCORES            # 128 out-features per core (2 heads)
NKT = D // 128              # 8 contraction tiles
NJT = S // 128              # 16 key tiles per batch
NICH = S // 512             # 4 query chunks per batch
NTT = S // 128              # 16 token tiles per batch


def build_program():
    nc = bacc.Bacc("TRN2", target_bir_lowering=False, debug=False,
                   num_devices=NCORES)
    xT_d = nc.dram_tensor("xT", [B, D, S], BF16, kind="ExternalInput")
    # host pre-arranged [128, NKT*OF] so each partition row is contiguous
    wq_d = nc.dram_tensor("wq", [128, NKT * OF], BF16, kind="ExternalInput")
    wk_d = nc.dram_tensor("wk", [128, NKT * OF], BF16, kind="ExternalInput")
    wv_d = nc.dram_tensor("wv", [128, NKT * OF], BF16, kind="ExternalInput")
    bqkv_d = nc.dram_tensor("bqkv", [OF, 3], F32, kind="ExternalInput")
    wo_d = nc.dram_tensor("wo", [OF, D], BF16, kind="ExternalInput")
    # exp(mask) per key position, duplicated x2 for the two-head ones column
    emask_d = nc.dram_tensor("emask", [128, B * NJT, 2], F32,
                             kind="ExternalInput")
    out_d = nc.dram_tensor("out", [B * S, D], BF16, kind="ExternalOutput")

    with tile.TileContext(nc) as tc:
        with (
            tc.tile_pool(name="consts", bufs=1) as consts,
            tc.tile_pool(name="xin", bufs=3) as xin,
            tc.tile_pool(name="qkv", bufs=2) as qkv,
            tc.tile_pool(name="attn", bufs=4) as attn,
            tc.tile_pool(name="ctxp", bufs=2) as ctxp,
            tc.tile_pool(name="outp", bufs=4) as outp,
            tc.tile_pool(name="psum", bufs=2, space="PSUM") as psum,
        ):
            # ---------------- constants ----------------
            ident = consts.tile([128, 128], BF16)
            make_identity(nc, ident)
            # spread const DMAs across issue queues: sync carries only wq +
            # the first xt chunk, so the first QKV chain starts ASAP
            w_sb = {}
            for nm, d, eng in (("q", wq_d, nc.sync), ("k", wk_d, nc.gpsimd),
                               ("v", wv_d, nc.gpsimd)):
                t = consts.tile([128, NKT, OF], BF16, name=f"w{nm}_sb")
                eng.dma_start(
                    t, d[:, :].rearrange("p (k f) -> p k f", k=NKT))
                w_sb[nm] = t
            bqkv_sb = consts.tile([OF, 3], F32)
            nc.gpsimd.dma_start(bqkv_sb, bqkv_d[:, :])
            b_sb = {nm: bqkv_sb[:, i:i + 1]
                    for i, nm in enumerate(("q", "k", "v"))}
            # issue late-needed consts from the gpsimd queue so the first
            # xt chunk isn't queued behind them
            emask_sb = consts.tile([128, B * NJT, 2], F32)
            nc.gpsimd.dma_start(emask_sb, emask_d[:, :, :])
            wo_sb = consts.tile([OF, D], BF16)
            nc.gpsimd.dma_start(wo_sb, wo_d[:, :])

            state = [dict() for _ in range(B)]

            def qkv_units(b):
                """QKV projections + V transpose for batch b. Each name's
                8-matmul accumulation chain is emitted unbroken so
                consecutive LDWEIGHTS pull ahead into the background
                weight buffer."""
                st = state[b]
                pT = {nm: qkv.tile([OF, S], BF16, name=f"{nm}T")
                      for nm in ("q", "k", "v")}
                st["pT"] = pT
                # [128 kpos, jt, h, 65]: per-head v tile + ones col 64
                va = qkv.tile([128, NJT, 2, 65], BF16, name="v_aug")
                st["va"] = va
                def transposes(nch):
                    for jt in range(4 * nch, 4 * nch + 4):
                        col = b * NJT + jt
                        # ones column carries exp(mask) so the denominator
                        # and PV weights absorb the additive mask
                        nc.vector.tensor_copy(
                            va[:, jt, :, 64], emask_sb[:, col, :])
                        pvt = psum.tile([128, 128], BF16, tag="mm",
                                        name="pvt")
                        nc.tensor.transpose(
                            pvt, pT["v"][:, ts(jt, 128)], ident)
                        # both heads in one 3-dim-AP copy, scaled by exp(mask)
                        # per key position (partition)
                        nc.vector.tensor_scalar_mul(
                            va[:, jt, :, 0:64],
                            pvt[:, :].rearrange("p (h f) -> p h f", h=2),
                            emask_sb[:, col, 0:1])

                for nch in range(NICH):
                    xt = xin.tile([128, NKT, 512], BF16, name="xt")
                    src = xT_d[b].rearrange("(k p) t -> p k t", p=128)
                    for half in range(2):
                        ks = slice(half * 4, half * 4 + 4)
                        nc.sync.dma_start(
                            xt[:, ks, :], src[:, ks, ts(nch, 512)])
                    yield
                    for nm in ("q", "k", "v"):
                        ps = psum.tile([128, 512], F32, tag="mm",
                                       name="ps_qkv")
                        for kt in range(NKT):
                            nc.tensor.matmul(
                                ps, lhsT=w_sb[nm][:, kt, :],
                                rhs=xt[:, kt, :],
                                start=(kt == 0), stop=(kt == NKT - 1),
                            )
                        nc.vector.tensor_scalar_add(
                            pT[nm][:, ts(nch, 512)], ps, b_sb[nm])
                        yield
                        # transposes of the previous chunk, one pipeline
                        # stage behind the DVE copy that feeds them
                        if nm == "k" and nch > 0:
                            transposes(nch - 1)
                            yield
                transposes(NICH - 1)

            def attn_units(b):
                """Attention + softmax normalize for batch b."""
                st = state[b]
                qT, kT = st["pT"]["q"], st["pT"]["k"]
                va = st["va"]
                ctxT = ctxp.tile([128, S], BF16, name="ctxT")
                st["ctxT"] = ctxT

                def phase_a(ich):
                    """scores + exp per j-tile (generator). The mask is
                    folded into v_aug multiplicatively, so the ACT op
                    needs no bias operand."""
                    isl = ts(ich, 512)
                    ets = []
                    for jt in range(NJT):
                        sc = psum.tile([128, 1024], F32, tag="sc", name="sc")
                        for h in range(2):
                            hs = slice(h * 64, (h + 1) * 64)
                            nc.tensor.matmul(
                                sc[:, ts(h, 512)],
                                lhsT=kT[hs, ts(jt, 128)], rhs=qT[hs, isl],
                                start=True, stop=True,
                            )
                        et = attn.tile([128, 1024], BF16, name="et",
                                       bufs=36)
                        nc.scalar.activation(et, sc, AF.Exp, scale=0.125)
                        ets.append((et, 0))
                        yield ets

                def phase_b(ich, ets):
                    """PV chains + normalize + (last batch) outproj."""
                    isl = ts(ich, 512)
                    pc = [psum.tile([65, 512], F32, tag="pc", name=f"pc{h}")
                          for h in range(2)]
                    for h in range(2):
                        for jt in range(NJT):
                            et, off = ets[jt]
                            o = off + h * 512
                            nc.tensor.matmul(
                                pc[h], lhsT=va[:, jt, h, :],
                                rhs=et[:, o:o + 512],
                                start=(jt == 0), stop=(jt == NJT - 1),
                            )
                        yield
                        # normalize h right away: h0's divide overlaps h1's
                        # PV chain, so only h1's sits on the critical path
                        den = attn.tile([1, 512], F32, name=f"den{h}")
                        nc.vector.tensor_copy(den, pc[h][64:65, :])
                        rec = attn.tile([1, 512], F32, name=f"rec{h}")
                        nc.vector.reciprocal_approx_fast(rec, den)
                        rep = attn.tile([64, 512], F32, name=f"rep{h}")
                        nc.gpsimd.partition_broadcast(rep, rec)
                        nc.vector.tensor_mul(
                            ctxT[h * 64:(h + 1) * 64, isl],
                            pc[h][0:64, :], rep)
                        yield
                    if b == B - 1:
                        # final batch: emit this chunk's output projection
                        # immediately so the epilogue is only 4 token-tiles
                        for tt in range(4 * ich, 4 * ich + 4):
                            outproj_tile(b, tt)
                            yield

                # software pipeline: phase A of chunk k+1 interleaves with
                # phase B of chunk k so the ACT engine is never starved
                ga = phase_a(0)
                ets = None
                for _ in ga:
                    ets = _
                    yield
                for ich in range(NICH):
                    gb = phase_b(ich, ets)
                    ga = phase_a(ich + 1) if ich + 1 < NICH else None
                    nets = None
                    live = True
                    while live:
                        live = False
                        if ga is not None:
                            for _ in range(4):
                                try:
                                    nets = next(ga)
                                    live = True
                                    yield
                                except StopIteration:
                                    break
                        try:
                            next(gb)
                            live = True
                            yield
                        except StopIteration:
                            pass
                    ets = nets

            def outproj_tile(b, tt):
                ctxT = state[b]["ctxT"]
                osb = outp.tile([128, D], BF16, name="osb", bufs=6)
                for oc in range(2):
                    po = psum.tile([128, 512], F32, tag="mm", name="po")
                    nc.tensor.matmul(
                        po, lhsT=ctxT[:, ts(tt, 128)],
                        rhs=wo_sb[:, ts(oc, 512)],
                        start=True, stop=True,
                    )
                    nc.vector.tensor_copy(osb[:, ts(oc, 512)], po)
                # single DMA per token tile, alternating issue queues so
                # output writes spread across DGE rings
                eng = nc.gpsimd if tt % 2 == 0 else nc.sync
                eng.dma_start(
                    out_d[b * S + tt * 128: b * S + (tt + 1) * 128, :], osb)

            def outproj_units(b):
                """Output projection for batch b (bf16 partial, no bias)."""
                for tt in range(NTT):
                    outproj_tile(b, tt)
                    yield

            def drain(*weighted):
                """weighted: (gen, stride[, delay]) — advance gen every
                `stride` cycles after `delay` cycles. Run until exhausted."""
                live = []
                for w in weighted:
                    g, s, d = (w + (0,)) if len(w) == 2 else w
                    if g is not None:
                        live.append((g, s, d))
                cyc = 0
                while live:
                    nxt = []
                    for g, s, d in live:
                        if cyc >= d and (cyc - d) % s == 0:
                            try:
                                next(g)
                            except StopIteration:
                                continue
                        nxt.append((g, s, d))
                    live = nxt
                    cyc += 1

            def pull(g, n):
                for _ in range(n):
                    try:
                        next(g)
                    except StopIteration:
                        return False
                return True

            g_attn = [attn_units(b) for b in range(B)]
            g_qkv = [qkv_units(b) for b in range(B)]
            g_out = [outproj_units(b) for b in range(B)]

            # prologue: batch 0 QKV fully paced in first, then attention(0)
            # starts while the rest of QKV(0) streams
            pull(g_qkv[0], 9)
            drain((g_attn[0], 2), (g_qkv[0], 1))
            for b in range(B):
                drain(
                    (g_attn[b], 1),
                    (g_qkv[b + 1] if b + 1 < B else None, 2 if b == 0 else 3),
                    (g_out[b - 1] if b >= 1 else None, 2),
                )
    nc.finalize()
    return nc


def make_in_maps(x, attention_mask, Wq, bq, Wk, bk, Wv, bv, Wo, bo):
    x = np.asarray(x, dtype=np.float32)
    attention_mask = np.asarray(attention_mask, dtype=np.float32)
    Wq, Wk, Wv, Wo = (np.asarray(a, dtype=np.float32) for a in (Wq, Wk, Wv, Wo))
    bq, bk, bv = (np.asarray(a, dtype=np.float32) for a in (bq, bk, bv))

    xT = np.ascontiguousarray(x.transpose(0, 2, 1)).astype(BF)  # [B, D, S]
    # exp(mask[b,0,0,j]) -> [128 partitions, B*NJT, 2] (dup for ones col)
    m = attention_mask.reshape(B, S).reshape(B, NJT, 128)
    em = np.exp(m.transpose(2, 0, 1).reshape(128, B * NJT))
    emask_host = np.ascontiguousarray(
        np.repeat(em[:, :, None], 2, axis=2)).astype(np.float32)

    def wlayout(W, cs):
        # [D, OF] -> [128 partitions, NKT*OF], partition rows contiguous
        wc = W[:, cs].reshape(NKT, 128, OF).transpose(1, 0, 2)
        return np.ascontiguousarray(wc.reshape(128, NKT * OF)).astype(BF)

    in_maps = []
    for c in range(NCORES):
        cs = slice(c * OF, (c + 1) * OF)
        in_maps.append({
            "xT": xT,
            "wq": wlayout(Wq, cs),
            "wk": wlayout(Wk, cs),
            "wv": wlayout(Wv, cs),
            "bqkv": np.ascontiguousarray(
                np.stack([bq[cs], bk[cs], bv[cs]], axis=1)),
            "wo": np.ascontiguousarray(Wo[cs, :]).astype(BF),
            "emask": emask_host,
        })
    return in_maps


_BO = [None]


def combine_outputs(results):
    acc = np.zeros((B * S, D), dtype=np.float64)
    for r in results:
        acc += r["out"].astype(np.float64)
    if _BO[0] is not None:
        acc += _BO[0]
    return acc.reshape(B, S, D).astype(np.float32)


_NC_CACHE = []


def _get_program():
    if not _NC_CACHE:
        _NC_CACHE.append(build_program())
    return _NC_CACHE[0]


def kernel(**inputs):
    nc = _get_program()
    _BO[0] = np.asarray(inputs["bo"], dtype=np.float64)
    in_maps = make_in_maps(**inputs)
    res = bass_utils.run_bass_kernel_spmd(
        nc, in_maps, core_ids=list(range(NCORES)))
    return combine_outputs(res.results)
